# revision 7
# baseline (speedup 1.0000x reference)
"""TRN2 Bass kernel for nn_DAGLayer (gnn_message_passing).

DAG of 1x1 convs over [B=64, C=64, H=32, W=32]:
  preproc: s0 = W_pre[0] @ x0, s1 = W_pre[1] @ x1   (channel matmul)
  node i (i=0..3): s_{2+i} = sum_j conv1x1(relu(s_j), W_edge[...]) over all
  prior states j; output = concat(s2..s5) on channels -> [B, 256, H, W].

Strategy: data-parallel over batch across 8 NeuronCores (8 batches/core).
Every 1x1 conv is a channel-dim matmul over N = H*W spatial columns.
Matmul operands are fp16 (10-bit mantissa, ~3e-4 per-matmul rel error,
full 1 cyc/row PE rate + fast weight load); accumulation stays fp32 in
PSUM and the raw node states are written out in fp32. Weights are packed
host-side into a single [128, 640] lhsT block so that:
  - preproc is one K=128 matmul (block-diag W_pre) on X = [x0_b; x1_b]
  - each node pair shares K=128 matmuls (two 64-ch states stacked)
  - the K=64 "second-tier" edges (r2->s3, r4->s5) run as M=64 matmuls
    into PSUM partitions 64:128 (tile_position=(0, 64)).
"""
import sys

sys.path.insert(0, '/opt/trn_rl_repo')

import numpy as np

N_CORES = 8
B, C, H, W_SP = 64, 64, 32, 32
BP = B // N_CORES          # batches per core
HW = H * W_SP              # 1024 spatial columns per batch
NCOL = 512                 # matmul free-dim tile (one fp32 PSUM bank)

# Set by test harness to capture an NTFF trace; harmless default.
TRACE = False
LAST_RESULTS = None

_cache = {}


def _pack_weights(W_pre: np.ndarray, W_edge: np.ndarray) -> np.ndarray:
    """Pack all conv weights into one [128, 640] fp16 lhsT block.

    Layout (cols):
      0:128   WP  block-diag preproc: out [s0; s1] from rhs [x0; x1]
      128:256 A1  out [s2 | s3p] from rhs R01 = [r0; r1]
      256:384 B1  out [s4p | s5p] from rhs R01
      384:512 B2  out [s4p | s5p] from rhs R23 = [r2; r3]
      512:576 A2  (rows 0:64) edge r2->s3, written at PSUM partitions 64:128
      576:640 B3  (rows 0:64) edge r4->s5, written at PSUM partitions 64:128
    lhsT[k, m] = W[m, k] (pre-transposed for the PE's stationary operand).
    """
    Wt = np.zeros((128, 640), np.float32)
    T = lambda w: np.ascontiguousarray(w.T)
    Wt[0:64, 0:64] = T(W_pre[0])
    Wt[64:128, 64:128] = T(W_pre[1])
    # A1: cols 0:64 -> s2 (edges 0(r0), 1(r1)); cols 64:128 -> s3p (2, 3)
    Wt[0:64, 128:192] = T(W_edge[0])
    Wt[64:128, 128:192] = T(W_edge[1])
    Wt[0:64, 192:256] = T(W_edge[2])
    Wt[64:128, 192:256] = T(W_edge[3])
    # B1: cols 0:64 -> s4p (5(r0), 6(r1)); cols 64:128 -> s5p (9, 10)
    Wt[0:64, 256:320] = T(W_edge[5])
    Wt[64:128, 256:320] = T(W_edge[6])
    Wt[0:64, 320:384] = T(W_edge[9])
    Wt[64:128, 320:384] = T(W_edge[10])
    # B2 (rhs [r2; r3]): cols 0:64 -> s4p (7(r2), 8(r3)); cols 64:128 -> s5p (11, 12)
    Wt[0:64, 384:448] = T(W_edge[7])
    Wt[64:128, 384:448] = T(W_edge[8])
    Wt[0:64, 448:512] = T(W_edge[11])
    Wt[64:128, 448:512] = T(W_edge[12])
    # second-tier edges (K=64, weights at rows 0:64)
    Wt[0:64, 512:576] = T(W_edge[4])
    Wt[0:64, 576:640] = T(W_edge[13])
    return Wt.astype(np.float16)


def _build_program():
    import concourse.tile as tile
    from concourse import bacc, mybir

    F16, F32 = mybir.dt.float16, mybir.dt.float32
    Relu = mybir.ActivationFunctionType.Relu

    nc = bacc.Bacc()
    X = nc.dram_tensor("X", [BP, 128, HW], F16, kind="ExternalInput")
    Wt = nc.dram_tensor("Wt", [128, 640], F16, kind="ExternalInput")
    O = nc.dram_tensor("O", [BP, 256, HW], F32, kind="ExternalOutput")

    with tile.TileContext(nc) as tc:
        with tc.tile_pool(name="wpool", bufs=1) as wpool, \
             tc.tile_pool(name="xpool", bufs=3) as xpool, \
             tc.tile_pool(name="rpool", bufs=3) as rpool, \
             tc.tile_pool(name="opool", bufs=2) as opool, \
             tc.tile_pool(name="ppool", bufs=2, space="PSUM") as ppool, \
             tc.tile_pool(name="apool", bufs=3, space="PSUM") as apool, \
             tc.tile_pool(name="bpool", bufs=3, space="PSUM") as bpool:
            w = wpool.tile([128, 640], F16, tag="w")
            nc.sync.dma_start(w[:], Wt[:])
            for it in range(BP * HW // NCOL):
                b, half = divmod(it, HW // NCOL)
                s = slice(half * NCOL, (half + 1) * NCOL)
                if half == 0:
                    x = xpool.tile([128, HW], F16, tag="x")
                    nc.sync.dma_start(x[:], X[b])
                    outA = opool.tile([128, HW], F32, tag="outA")
                    outB = opool.tile([128, HW], F32, tag="outB")

                # preproc: [s0; s1]
                pP = ppool.tile([128, NCOL], F32, tag="pP")
                nc.tensor.matmul(pP[:], w[:, 0:128], x[:, s],
                                 start=True, stop=True)
                r01 = rpool.tile([128, NCOL], F16, tag="r01")
                nc.scalar.activation(r01[:], pP[:], Relu)

                # node pair A: pA = [s2; s3]
                pA = apool.tile([128, NCOL], F32, tag="pA")
                nc.tensor.matmul(pA[:], w[:, 128:256], r01[:],
                                 start=True, stop=False)
                r23 = rpool.tile([128, NCOL], F16, tag="r23")
                nc.scalar.activation(r23[0:64, :], pA[0:64, :], Relu)      # r2
                # node pair B partials that only need R01 (keeps PE busy
                # while the r2 relu runs)
                pB = bpool.tile([128, NCOL], F32, tag="pB")
                nc.tensor.matmul(pB[:], w[:, 256:384], r01[:],
                                 start=True, stop=False)
                # second-tier edge r2 -> s3 into PSUM partitions 64:128
                nc.tensor.matmul(pA[64:128, :], w[0:64, 512:576],
                                 r23[0:64, :], start=False, stop=True,
                                 tile_position=(0, 64))
                nc.vector.tensor_relu(r23[64:128, :], pA[64:128, :])       # r3
                nc.vector.tensor_copy(outA[:, s], pA[:])
                nc.tensor.matmul(pB[:], w[:, 384:512], r23[:],
                                 start=False, stop=False)
                r4 = rpool.tile([128, NCOL], F16, tag="r4")
                nc.scalar.activation(r4[0:64, :], pB[0:64, :], Relu)       # r4
                nc.tensor.matmul(pB[64:128, :], w[0:64, 576:640],
                                 r4[0:64, :], start=False, stop=True,
                                 tile_position=(0, 64))
                nc.vector.tensor_copy(outB[:, s], pB[:])

                # output channel order: s2 | s3 | s4 | s5
                if half == HW // NCOL - 1:
                    nc.sync.dma_start(O[b, 0:128, :], outA[:])
                    nc.sync.dma_start(O[b, 128:256, :], outB[:])
    nc.compile()
    return nc


def _get_program():
    if "nc" not in _cache:
        _cache["nc"] = _build_program()
    return _cache["nc"]


def kernel(x0, x1, W_pre, W_edge):
    global LAST_RESULTS
    from concourse.bass_utils import run_bass_kernel_spmd

    nc = _get_program()
    Xp = np.concatenate(
        [x0.reshape(B, C, HW), x1.reshape(B, C, HW)], axis=1)   # [B, 128, HW]
    Xp = Xp.astype(np.float16)
    Wt = _pack_weights(np.asarray(W_pre, np.float32), np.asarray(W_edge, np.float32))
    in_maps = [
        {"X": np.ascontiguousarray(Xp[i * BP:(i + 1) * BP]), "Wt": Wt}
        for i in range(N_CORES)
    ]
    res = run_bass_kernel_spmd(nc, in_maps, core_ids=list(range(N_CORES)),
                               trace=TRACE)
    LAST_RESULTS = res
    out = np.concatenate([r["O"] for r in res.results], axis=0)  # [B, 256, HW]
    return np.ascontiguousarray(out.reshape(B, 4 * C, H, W_SP))


# revision 8
# speedup vs baseline: 1.0926x; 1.0926x over previous
"""TRN2 Bass kernel for nn_DAGLayer (gnn_message_passing).

DAG of 1x1 convs over [B=64, C=64, H=32, W=32]:
  preproc: s0 = W_pre[0] @ x0, s1 = W_pre[1] @ x1   (channel matmul)
  node i (i=0..3): s_{2+i} = sum_j conv1x1(relu(s_j), W_edge[...]) over all
  prior states j; output = concat(s2..s5) on channels -> [B, 256, H, W].

Strategy: data-parallel over batch across 8 NeuronCores (8 batches/core).
Every 1x1 conv is a channel-dim matmul over N = H*W spatial columns.
Matmul operands are fp16 (10-bit mantissa, ~3e-4 per-matmul rel error,
full 1 cyc/row PE rate + fast weight load); accumulation stays fp32 in
PSUM and the raw node states are written out in fp32. Weights are packed
host-side into a single [128, 640] lhsT block so that:
  - preproc is one K=128 matmul (block-diag W_pre) on X = [x0_b; x1_b]
  - each node pair shares K=128 matmuls (two 64-ch states stacked)
  - the K=64 "second-tier" edges (r2->s3, r4->s5) run as M=64 matmuls
    into PSUM partitions 64:128 (tile_position=(0, 64)).
"""
import sys

sys.path.insert(0, '/opt/trn_rl_repo')

import numpy as np

N_CORES = 8
B, C, H, W_SP = 64, 64, 32, 32
BP = B // N_CORES          # batches per core
HW = H * W_SP              # 1024 spatial columns per batch
NCOL = 512                 # matmul free-dim tile (one fp32 PSUM bank)

# Set by test harness to capture an NTFF trace; harmless default.
TRACE = False
LAST_RESULTS = None

_cache = {}


def _pack_weights(W_pre: np.ndarray, W_edge: np.ndarray) -> np.ndarray:
    """Pack all conv weights into one [128, 640] fp16 lhsT block.

    Layout (cols):
      0:128   WP  block-diag preproc: out [s0; s1] from rhs [x0; x1]
      128:256 A1  out [s2 | s3p] from rhs R01 = [r0; r1]
      256:384 B1  out [s4p | s5p] from rhs R01
      384:512 B2  out [s4p | s5p] from rhs R23 = [r2; r3]
      512:576 A2  (rows 0:64) edge r2->s3, written at PSUM partitions 64:128
      576:640 B3  (rows 0:64) edge r4->s5, written at PSUM partitions 64:128
    lhsT[k, m] = W[m, k] (pre-transposed for the PE's stationary operand).
    """
    Wt = np.zeros((128, 640), np.float32)
    T = lambda w: np.ascontiguousarray(w.T)
    Wt[0:64, 0:64] = T(W_pre[0])
    Wt[64:128, 64:128] = T(W_pre[1])
    # A1: cols 0:64 -> s2 (edges 0(r0), 1(r1)); cols 64:128 -> s3p (2, 3)
    Wt[0:64, 128:192] = T(W_edge[0])
    Wt[64:128, 128:192] = T(W_edge[1])
    Wt[0:64, 192:256] = T(W_edge[2])
    Wt[64:128, 192:256] = T(W_edge[3])
    # B1: cols 0:64 -> s4p (5(r0), 6(r1)); cols 64:128 -> s5p (9, 10)
    Wt[0:64, 256:320] = T(W_edge[5])
    Wt[64:128, 256:320] = T(W_edge[6])
    Wt[0:64, 320:384] = T(W_edge[9])
    Wt[64:128, 320:384] = T(W_edge[10])
    # B2 (rhs [r2; r3]): cols 0:64 -> s4p (7(r2), 8(r3)); cols 64:128 -> s5p (11, 12)
    Wt[0:64, 384:448] = T(W_edge[7])
    Wt[64:128, 384:448] = T(W_edge[8])
    Wt[0:64, 448:512] = T(W_edge[11])
    Wt[64:128, 448:512] = T(W_edge[12])
    # second-tier edges (K=64, weights at rows 0:64)
    Wt[0:64, 512:576] = T(W_edge[4])
    Wt[0:64, 576:640] = T(W_edge[13])
    return Wt.astype(np.float16)


def _build_program():
    import concourse.tile as tile
    from concourse import bacc, mybir

    F16, F32 = mybir.dt.float16, mybir.dt.float32
    Relu = mybir.ActivationFunctionType.Relu

    nc = bacc.Bacc()
    X = nc.dram_tensor("X", [BP, 128, HW], F16, kind="ExternalInput")
    Wt = nc.dram_tensor("Wt", [128, 640], F16, kind="ExternalInput")
    O = nc.dram_tensor("O", [BP, 256, HW], F32, kind="ExternalOutput")

    with tile.TileContext(nc) as tc:
        with tc.tile_pool(name="wpool", bufs=1) as wpool, \
             tc.tile_pool(name="xpool", bufs=3) as xpool, \
             tc.tile_pool(name="rpool", bufs=3) as rpool, \
             tc.tile_pool(name="opool", bufs=3) as opool, \
             tc.tile_pool(name="ppool", bufs=2, space="PSUM") as ppool, \
             tc.tile_pool(name="apool", bufs=3, space="PSUM") as apool, \
             tc.tile_pool(name="bpool", bufs=3, space="PSUM") as bpool:
            w = wpool.tile([128, 640], F16, tag="w")
            nc.sync.dma_start(w[:], Wt[:])
            for it in range(BP * HW // NCOL):
                b, half = divmod(it, HW // NCOL)
                s = slice(half * NCOL, (half + 1) * NCOL)
                if half == 0:
                    x = xpool.tile([128, HW], F16, tag="x")
                    nc.sync.dma_start(x[:], X[b])
                    outA = opool.tile([128, HW], F32, tag="outA")
                    outB = opool.tile([128, HW], F32, tag="outB")

                # preproc: [s0; s1]
                pP = ppool.tile([128, NCOL], F32, tag="pP")
                nc.tensor.matmul(pP[:], w[:, 0:128], x[:, s],
                                 start=True, stop=True)
                r01 = rpool.tile([128, NCOL], F16, tag="r01")
                nc.scalar.activation(r01[:], pP[:], Relu)

                # node pair A: pA = [s2; s3]
                pA = apool.tile([128, NCOL], F32, tag="pA")
                nc.tensor.matmul(pA[:], w[:, 128:256], r01[:],
                                 start=True, stop=False)
                r23 = rpool.tile([128, NCOL], F16, tag="r23")
                nc.scalar.activation(r23[0:64, :], pA[0:64, :], Relu)      # r2
                # node pair B partials that only need R01 (keeps PE busy
                # while the r2 relu runs)
                pB = bpool.tile([128, NCOL], F32, tag="pB")
                nc.tensor.matmul(pB[:], w[:, 256:384], r01[:],
                                 start=True, stop=False)
                # second-tier edge r2 -> s3 into PSUM partitions 64:128
                nc.tensor.matmul(pA[64:128, :], w[0:64, 512:576],
                                 r23[0:64, :], start=False, stop=True,
                                 tile_position=(0, 64))
                nc.vector.tensor_relu(r23[64:128, :], pA[64:128, :])       # r3
                nc.vector.tensor_copy(outA[:, s], pA[:])
                nc.tensor.matmul(pB[:], w[:, 384:512], r23[:],
                                 start=False, stop=False)
                r4 = rpool.tile([128, NCOL], F16, tag="r4")
                nc.scalar.activation(r4[0:64, :], pB[0:64, :], Relu)       # r4
                nc.tensor.matmul(pB[64:128, :], w[0:64, 576:640],
                                 r4[0:64, :], start=False, stop=True,
                                 tile_position=(0, 64))
                nc.vector.tensor_copy(outB[:, s], pB[:])

                # output channel order: s2 | s3 | s4 | s5
                if half == HW // NCOL - 1:
                    nc.sync.dma_start(O[b, 0:128, :], outA[:])
                    nc.sync.dma_start(O[b, 128:256, :], outB[:])
    nc.compile()
    return nc


def _get_program():
    if "nc" not in _cache:
        _cache["nc"] = _build_program()
    return _cache["nc"]


def kernel(x0, x1, W_pre, W_edge):
    global LAST_RESULTS
    from concourse.bass_utils import run_bass_kernel_spmd

    nc = _get_program()
    Xp = np.concatenate(
        [x0.reshape(B, C, HW), x1.reshape(B, C, HW)], axis=1)   # [B, 128, HW]
    Xp = Xp.astype(np.float16)
    Wt = _pack_weights(np.asarray(W_pre, np.float32), np.asarray(W_edge, np.float32))
    in_maps = [
        {"X": np.ascontiguousarray(Xp[i * BP:(i + 1) * BP]), "Wt": Wt}
        for i in range(N_CORES)
    ]
    res = run_bass_kernel_spmd(nc, in_maps, core_ids=list(range(N_CORES)),
                               trace=TRACE)
    LAST_RESULTS = res
    out = np.concatenate([r["O"] for r in res.results], axis=0)  # [B, 256, HW]
    return np.ascontiguousarray(out.reshape(B, 4 * C, H, W_SP))
